# revision 1
# baseline (speedup 1.0000x reference)
"""PointPillarScatter on 8 NeuronCores.

Full inputs -> full (B, C, NX, NY) float32 output.

Sharding: core k handles (sample b = k//2, output-x half h = k%2); each core
produces out[b, :, h*216:(h+1)*216, :] (the flip along x is baked into the
host-built scatter offsets).

Per-core device pipeline, two phases:

  Phase 1 (sparse scatter, ~6k rows/core):
    The ~6k real pillar rows are DMA'd densely into SBUF and scattered by
    dma_scatter_add into a runtime-pre-zeroed DRAM staging canvas.  Staging is
    laid out partition-major: partition p owns 838 consecutive rows (837 canvas
    positions {i : i % 128 == p} ordered by i // 128, plus 1 dump row for the
    padding slots), so the offsets bake in both the scatter and the
    transpose-friendly permutation, and int16 offsets stay in range per
    32-partition region.

  Phase 2 (dense stream, memory-bound):
    Per chunk of 24 output-x rows: one big contiguous DMA pulls the staging
    slice into SBUF as [128 pos-in-block, 93 blocks, 64 ch]; PE transposes
    pairs of 128-position blocks through an identity ([128,128] -> PSUM);
    DVE/ACT copy PSUM into the [64 ch, 11904 pos] out tile; one 3 MB DMA
    writes the (C, X, Y) canvas slice.
"""

import sys

sys.path.insert(0, "/opt/trn_rl_repo")

import numpy as np

import concourse.bacc as bacc
import concourse.mybir as mybir
from concourse.bass_utils import run_bass_kernel_spmd
from concourse.masks import make_identity
from concourse.tile import TileContext

C = 64
NX = 432
NY = 496
B = 4
NCORES = 8
XH = NX // 2            # 216 x-rows per core
M = XH * NY             # 107136 positions per core
P = 128
JPP = M // P            # 837 rows of 128 positions per partition
XCHUNK = 8
NCHUNK = XH // XCHUNK   # 27
MC = XCHUNK * NY        # 3968 positions per chunk
JBLK = MC // P          # 31 blocks of 128 positions
CSPLITS = [0, 2, 9, 18, 27]             # chunk ranges per staging tensor
NSPLIT = len(CSPLITS) - 1
JS = [(CSPLITS[i + 1] - CSPLITS[i]) * JBLK for i in range(NSPLIT)]   # rows/partition
RPS = [j + 1 for j in JS]               # +1 dump row
NREG = 2                # int16 offsets cover 64 partitions x <=218 rows
PREG = P // NREG        # 64 partitions per region

_CACHE = {}
LAST_RESULTS = None


def _build_program(jr):
    nslot = P * jr          # scatter slots per segment (padded, fixed count)
    NSEG = NSPLIT * NREG    # (j-split, region)
    nc = bacc.Bacc(None, target_bir_lowering=False)
    feats = nc.dram_tensor("feats", [NSEG * nslot, C], mybir.dt.float32, kind="ExternalInput")
    sidx = nc.dram_tensor("sidx", [P, NSEG * nslot // 16], mybir.dt.int16, kind="ExternalInput")
    sts = [
        nc.dram_tensor(f"st{i}", [P * RPS[i], C], mybir.dt.float32, kind="ExternalOutput")
        for i in range(NSPLIT)
    ]
    out = nc.dram_tensor("out", [C, XH, NY], mybir.dt.float32, kind="ExternalOutput")

    views = [sts[i][:].rearrange("(pt j) c -> pt j c", j=RPS[i]) for i in range(NSPLIT)]

    with TileContext(nc) as tc:
        with (
            tc.tile_pool(name="scat", bufs=2) as scatp,
            tc.tile_pool(name="sidxp", bufs=2) as sidxp,
            tc.tile_pool(name="const", bufs=1) as constp,
            tc.tile_pool(name="gather", bufs=6) as gatherp,
            tc.tile_pool(name="outp", bufs=4) as outp,
            tc.tile_pool(name="psum", bufs=4, space="PSUM") as psump,
            tc.tile_pool(name="psums", bufs=2, space="PSUM") as psumsp,
        ):
            for seg in range(NSEG):
                sp, r = divmod(seg, NREG)
                regrows = PREG * RPS[sp]
                ft = scatp.tile([P, jr, C], mybir.dt.float32, tag="ft")
                nc.scalar.dma_start(ft[:], feats[seg * nslot:(seg + 1) * nslot, :].rearrange("(p j) c -> p j c", j=jr))
                it = sidxp.tile([P, nslot // 16], mybir.dt.int16, tag="it")
                nc.scalar.dma_start(it[:], sidx[:, seg * (nslot // 16):(seg + 1) * (nslot // 16)])
                nc.gpsimd.dma_scatter_add(
                    out_ap=sts[sp][r * regrows:(r + 1) * regrows, :],
                    in_ap=ft[:],
                    idxs_ap=it[:],
                    num_idxs=nslot,
                    num_idxs_reg=nslot,
                    elem_size=C,
                    single_packet=False,
                )

            ident = constp.tile([P, P], mybir.dt.float32)
            make_identity(nc, ident[:])

            for ci in range(NCHUNK):
                sp = next(i for i in range(NSPLIT) if CSPLITS[i] <= ci < CSPLITS[i + 1])
                cl = ci - CSPLITS[sp]
                src = views[sp][:, cl * JBLK:(cl + 1) * JBLK, :]
                gt = gatherp.tile([P, JBLK * C], mybir.dt.float32, tag="gt")
                nc.scalar.dma_start(gt[:].rearrange("p (j c) -> p j c", c=C), src)

                ot = outp.tile([C, MC], mybir.dt.float32, tag="ot")
                npairs = JBLK // 2
                nquads = (npairs + 3) // 4
                for q in range(nquads):
                    np_q = min(4, npairs - q * 4)
                    pt = psump.tile([P, 512], mybir.dt.float32, tag="pt")
                    for m in range(np_q):
                        k = q * 4 + m
                        nc.tensor.transpose(pt[:, m * P:(m + 1) * P], gt[:, k * P:(k + 1) * P], ident[:])
                    base = q * 4 * 2 * P
                    dst = ot[:, base:base + np_q * 2 * P].rearrange("c (n two x) -> c n two x", two=2, x=P)
                    src_ps = pt[:, :np_q * P]
                    nc.vector.tensor_copy(dst[:, :, 0, :], src_ps[0:C, :].rearrange("c (n x) -> c n x", x=P))
                    nc.scalar.copy(dst[:, :, 1, :], src_ps[C:P, :].rearrange("c (n x) -> c n x", x=P))
                j = JBLK - 1
                pt = psumsp.tile([P, P], mybir.dt.float32, tag="pts")
                nc.tensor.transpose(pt[0:C, :], gt[:, j * C:(j + 1) * C], ident[:])
                nc.vector.tensor_copy(ot[:, j * P:(j + 1) * P], pt[0:C, :])
                nc.sync.dma_start(out[:, ci * XCHUNK:(ci + 1) * XCHUNK, :], ot[:].rearrange("c (x y) -> c x y", y=NY))

    nc.finalize()
    return nc


def _prep_in_maps(feats_full, batch_indices, sample_indices):
    x = batch_indices[:, 2].astype(np.int64)
    y = batch_indices[:, 1].astype(np.int64)
    sm = sample_indices.astype(np.int64)
    xo = (NX - 1) - x
    h = xo // XH
    xl = xo % XH
    pos = xl * NY + y
    core = sm * 2 + h

    pp = pos % P            # partition
    jj = pos // P           # row within partition
    reg = pp // PREG

    jbounds = np.array([c * JBLK for c in CSPLITS])
    sp = np.searchsorted(jbounds, jj, side="right") - 1     # which staging tensor
    rp_arr = np.array(RPS)[sp]
    jloc = jj - jbounds[sp]
    seg = sp * NREG + reg
    local = (pp % PREG) * rp_arr + jloc                     # int16-safe

    NSEG = NSPLIT * NREG
    maxn = 0
    for k in range(NCORES):
        for g in range(NSEG):
            maxn = max(maxn, int(np.sum((core == k) & (seg == g))))
    jr = -(-(maxn + 1) // P) + 1     # ceil to 128 slots + 1 spare column

    nslot = P * jr
    in_maps = []
    for k in range(NCORES):
        feats_arr = np.zeros((NSEG * nslot, C), np.float32)
        idx_arr = np.full((16, NSEG * nslot // 16), 0, np.int16)
        for g in range(NSEG):
            sp_g = g // NREG
            rows = np.nonzero((core == k) & (seg == g))[0]
            loc = local[rows]
            order = np.argsort(loc)
            rows = rows[order]
            loc = loc[order]
            n = rows.size
            assert n <= nslot
            slots = np.arange(nslot)
            vals = np.full(nslot, 0, np.int16)
            vals[:n] = loc.astype(np.int16)
            vals[n:] = ((slots[n:] % P) % PREG) * RPS[sp_g] + JS[sp_g]  # dump row
            d = (slots[:n] % P) * jr + slots[:n] // P
            feats_arr[g * nslot + d] = feats_full[rows]
            idx_arr[:, g * (nslot // 16):(g + 1) * (nslot // 16)] = vals.reshape(nslot // 16, 16).T
        in_maps.append({"feats": feats_arr, "sidx": np.ascontiguousarray(np.tile(idx_arr, (8, 1)))})
    return in_maps, jr


def kernel(batch_pillar_features, batch_indices, sample_indices, batch_size):
    global LAST_RESULTS
    feats_full = np.asarray(batch_pillar_features, np.float32)
    batch_indices = np.asarray(batch_indices)
    sample_indices = np.asarray(sample_indices)
    bs = int(batch_size)
    assert bs == B and feats_full.shape[1] == C

    in_maps, jr = _prep_in_maps(feats_full, batch_indices, sample_indices)
    if _CACHE.get("jr") != jr:
        _CACHE["nc"] = _build_program(jr)
        _CACHE["jr"] = jr
    nc = _CACHE["nc"]

    res = run_bass_kernel_spmd(nc, in_maps, core_ids=list(range(NCORES)))
    LAST_RESULTS = res

    full = np.empty((B, C, NX, NY), np.float32)
    for k in range(NCORES):
        b, hh = k // 2, k % 2
        full[b, :, hh * XH:(hh + 1) * XH, :] = res.results[k]["out"]
    return full



# revision 5
# speedup vs baseline: 1.3204x; 1.3204x over previous
"""PointPillarScatter on 8 NeuronCores.

Full inputs -> full (B, C, NX, NY) float32 output.

Sharding: core k handles (sample b = k//2, output-x half h = k%2); each core
produces out[b, :, h*216:(h+1)*216, :] (the flip along x is baked into the
host-built scatter offsets).

Per-core device pipeline (no DRAM staging round-trip):

  The canvas lives in SBUF.  Per chunk of 24 output-x rows (MC = 11904
  positions = 93 blocks of 128):

  1. dma_scatter_add in SBUF-destination mode (sbuf_tokens_per_rank=128,
     all-even rank slots so out_ap_other aliases out_ap) scatters the ~750
     real pillar rows of the chunk into a pre-zeroed canvas tile
     A[128 part = pos%128, block g = pos//128, 64 ch].
  2. PE transposes pairs of blocks ([128 pos, 128=2x64 ch]) into 4-bank
     PSUM tiles (16 transposes per [128, 2048] tile); DVE copies the
     even-block rows (0:64) and ACT the odd-block rows (64:128, with the
     partition shift) into a contiguous ot[64 ch, MC] tile.
  3. Two half-chunk DMAs (SP / ACT queues) stream ot to the (C, X, Y)
     DRAM output with ~24 KB contiguous per-partition lines.
  4. Canvas re-zeroed for the next round by memsets spread across
     DVE / Pool / ACT (ACT zeroes by copying from a zero tile).

  HBM traffic per core is just feats in (~2 MB) + output out (27.4 MB),
  vs ~85 MB for a DRAM-staging design.
"""

import sys

sys.path.insert(0, "/opt/trn_rl_repo")

import numpy as np

import concourse.bacc as bacc
import concourse.mybir as mybir
from concourse.bass_utils import run_bass_kernel_spmd
from concourse.masks import make_identity
from concourse.tile import TileContext

C = 64
NX = 432
NY = 496
B = 4
NCORES = 8
XH = NX // 2            # 216 x-rows per core
M = XH * NY             # 107136 positions per core
P = 128
XCHUNK = 24
NCHUNK = XH // XCHUNK   # 9
MC = XCHUNK * NY        # 11904 positions per chunk
JBLK = MC // P          # 93 blocks of 128 positions

_CACHE = {}
LAST_RESULTS = None


def _build_program(jr):
    nslot = P * jr
    nc = bacc.Bacc(None, target_bir_lowering=False)
    feats = nc.dram_tensor("feats", [NCHUNK * nslot, C], mybir.dt.float32, kind="ExternalInput")
    sidx = nc.dram_tensor("sidx", [P, NCHUNK * nslot // 16], mybir.dt.int16, kind="ExternalInput")
    out = nc.dram_tensor("out", [C, XH, NY], mybir.dt.float32, kind="ExternalOutput")
    out_flat = out[:].rearrange("c x y -> c (x y)")

    with TileContext(nc) as tc:
        with (
            tc.tile_pool(name="io", bufs=2) as iop,
            tc.tile_pool(name="idx", bufs=2) as idxp,
            tc.tile_pool(name="canvas", bufs=1) as canp,
            tc.tile_pool(name="ot", bufs=2) as otp,
            tc.tile_pool(name="const", bufs=1) as constp,
            tc.tile_pool(name="psum", bufs=2, space="PSUM") as psump,
        ):
            ident = constp.tile([P, P], mybir.dt.float32)
            make_identity(nc, ident[:])
            zsrc = constp.tile([P, 1536], mybir.dt.float32)
            nc.vector.memset(zsrc[:], 0.0)

            canvases = []
            for bu in range(2):
                Ab = canp.tile([P, JBLK * C], mybir.dt.float32, tag=f"A{bu}")
                nc.vector.memset(Ab[:], 0.0)
                canvases.append(Ab)

            for ci in range(NCHUNK):
                A = canvases[ci % 2]
                ft = iop.tile([P, jr, C], mybir.dt.float32, tag="ft")
                nc.sync.dma_start(ft[:], feats[ci * nslot:(ci + 1) * nslot, :].rearrange("(p j) c -> p j c", p=P))
                it = idxp.tile([P, nslot // 16], mybir.dt.int16, tag="it")
                nc.sync.dma_start(it[:], sidx[:, ci * (nslot // 16):(ci + 1) * (nslot // 16)])

                nc.gpsimd.dma_scatter_add(
                    out_ap=A[:], in_ap=ft[:], idxs_ap=it[:],
                    num_idxs=nslot, num_idxs_reg=nslot, elem_size=C,
                    single_packet=False, sbuf_tokens_per_rank=P,
                    parity_reg=0, out_ap_other=A[:],
                )

                ot = otp.tile([C, MC], mybir.dt.float32, tag="ot")
                # 93 blocks: 2 full psum tiles of 16 pair-transposes (32 blocks
                # each), 1 tail tile of 14 pairs + 1 single (29 blocks).
                for t in range(3):
                    b0 = t * 32                       # first block of this tile
                    nb = min(32, JBLK - b0)           # blocks in tile (32/32/29)
                    npair = nb // 2
                    pt = psump.tile([P, 2048], mybir.dt.float32, tag="pt")
                    for m in range(npair):
                        g0 = b0 + 2 * m
                        nc.tensor.transpose(pt[:, m * P:(m + 1) * P], A[:, g0 * C:(g0 + 2) * C], ident[:])
                    if nb % 2:
                        nc.tensor.transpose(pt[0:C, npair * P:(npair + 1) * P], A[:, (b0 + nb - 1) * C:(b0 + nb) * C], ident[:])
                    dst = ot[:, b0 * P:(b0 + nb) * P].rearrange("c (k i) -> c k i", i=P)
                    ne, no = (nb + 1) // 2, nb // 2
                    nc.vector.tensor_copy(dst[:, 0::2, :], pt[0:C, 0:ne * P].rearrange("c (k i) -> c k i", i=P))
                    nc.scalar.copy(dst[:, 1::2, :], pt[C:P, 0:no * P].rearrange("c (k i) -> c k i", i=P))

                # out: two half-chunk DMAs on separate queues
                HMC = MC // 2
                nc.sync.dma_start(out_flat[:, ci * MC: ci * MC + HMC], ot[:, 0:HMC])
                nc.scalar.dma_start(out_flat[:, ci * MC + HMC:(ci + 1) * MC], ot[:, HMC:MC])

                # re-zero the canvas for chunk ci+2 (split across engines)
                nc.gpsimd.memset(A[:, 0:4416], 0.0)
                nc.scalar.copy(A[:, 4416:JBLK * C], zsrc[:])

    nc.finalize()
    return nc


def _prep_in_maps(feats_full, batch_indices, sample_indices):
    x = batch_indices[:, 2].astype(np.int64)
    y = batch_indices[:, 1].astype(np.int64)
    sm = sample_indices.astype(np.int64)
    xo = (NX - 1) - x
    h = xo // XH
    xl = xo % XH
    pos = xl * NY + y
    core = sm * 2 + h

    ci = pos // MC                  # chunk
    local = pos % MC
    idx16 = (local // P) * 256 + (local % P)

    grp = core * NCHUNK + ci
    counts = np.bincount(grp, minlength=NCORES * NCHUNK)
    maxn = int(counts.max())
    jr = -(-maxn // P)
    nslot = P * jr

    order = np.argsort(grp, kind="stable")
    in_maps = []
    off = 0
    for k in range(NCORES):
        feats_arr = np.zeros((NCHUNK * nslot, C), np.float32)
        idx_arr = np.zeros((P, NCHUNK * nslot // 16), np.int16)
        for g in range(NCHUNK):
            n = counts[k * NCHUNK + g]
            rows = order[off:off + n]
            off += n
            slots = np.arange(n)
            vals = np.zeros(nslot, np.int16)
            vals[:n] = idx16[rows].astype(np.int16)
            d = (slots % P) * jr + slots // P
            feats_arr[g * nslot + d] = feats_full[rows]
            idx_arr[:16, g * (nslot // 16):(g + 1) * (nslot // 16)] = vals.reshape(nslot // 16, 16).T
        idx_arr[16:] = np.tile(idx_arr[:16], (7, 1))
        in_maps.append({"feats": feats_arr, "sidx": idx_arr})
    return in_maps, jr


def kernel(batch_pillar_features, batch_indices, sample_indices, batch_size):
    global LAST_RESULTS
    feats_full = np.asarray(batch_pillar_features, np.float32)
    batch_indices = np.asarray(batch_indices)
    sample_indices = np.asarray(sample_indices)
    bs = int(batch_size)
    assert bs == B and feats_full.shape[1] == C

    in_maps, jr = _prep_in_maps(feats_full, batch_indices, sample_indices)
    if _CACHE.get("jr") != jr:
        _CACHE["nc"] = _build_program(jr)
        _CACHE["jr"] = jr
    nc = _CACHE["nc"]

    res = run_bass_kernel_spmd(nc, in_maps, core_ids=list(range(NCORES)))
    LAST_RESULTS = res

    full = np.empty((B, C, NX, NY), np.float32)
    for k in range(NCORES):
        b, hh = k // 2, k % 2
        full[b, :, hh * XH:(hh + 1) * XH, :] = res.results[k]["out"]
    return full


# revision 20
# speedup vs baseline: 1.3863x; 1.0499x over previous
"""PointPillarScatter on 8 NeuronCores.

Full inputs -> full (B, C, NX, NY) float32 output.

Sharding: core k handles (sample b = k//2, output-x half h = k%2); each core
produces out[b, :, h*216:(h+1)*216, :] (the flip along x is baked into the
host-built scatter offsets).

Per-core device pipeline (no DRAM staging round-trip):

  The canvas lives in SBUF.  Per chunk of 24 output-x rows (MC = 11904
  positions = 93 blocks of 128):

  1. dma_scatter_add in SBUF-destination mode (sbuf_tokens_per_rank=128,
     all-even rank slots so out_ap_other aliases out_ap) scatters the ~750
     real pillar rows of the chunk into a pre-zeroed canvas tile
     A[128 part = pos%128, block g = pos//128, 64 ch].
  2. PE transposes pairs of blocks ([128 pos, 128=2x64 ch]) into 4-bank
     PSUM tiles (16 transposes per [128, 2048] tile); DVE copies the
     even-block rows (0:64) and ACT the odd-block rows (64:128, with the
     partition shift) into a contiguous ot[64 ch, MC] tile.
  3. Two half-chunk DMAs (SP / ACT queues) stream ot to the (C, X, Y)
     DRAM output with ~24 KB contiguous per-partition lines.
  4. Canvas re-zeroed for the next round by memsets spread across
     DVE / Pool / ACT (ACT zeroes by copying from a zero tile).

  HBM traffic per core is just feats in (~2 MB) + output out (27.4 MB),
  vs ~85 MB for a DRAM-staging design.
"""

import sys

sys.path.insert(0, "/opt/trn_rl_repo")

import numpy as np

import concourse.bacc as bacc
import concourse.mybir as mybir
from concourse.bass_utils import run_bass_kernel_spmd
from concourse.masks import make_identity
from concourse.tile import TileContext

C = 64
NX = 432
NY = 496
B = 4
NCORES = 8
XH = NX // 2            # 216 x-rows per core
M = XH * NY             # 107136 positions per core
P = 128
XCHUNK = 24
NCHUNK = XH // XCHUNK   # 9
MC = XCHUNK * NY        # 11904 positions per chunk
JBLK = MC // P          # 93 blocks of 128 positions

_CACHE = {}
LAST_RESULTS = None

# STRIDE4_SWIZZLE port-rotation order; _SWZ_POS[p] = issue rank of partition p
_STRIDE4 = np.array([(i % 32) * 4 + (i // 32) for i in range(P)])
_SWZ_POS = np.empty(P, np.int64)
_SWZ_POS[_STRIDE4] = np.arange(P)


def _build_program(jr):
    nslot = P * jr
    nc = bacc.Bacc(None, target_bir_lowering=False)
    feats = nc.dram_tensor("feats", [NCHUNK * nslot, C], mybir.dt.float32, kind="ExternalInput")
    sidx = nc.dram_tensor("sidx", [P, NCHUNK * nslot // 16], mybir.dt.int16, kind="ExternalInput")
    out = nc.dram_tensor("out", [C, XH, NY], mybir.dt.float32, kind="ExternalOutput")
    out_flat = out[:].rearrange("c x y -> c (x y)")

    with TileContext(nc) as tc:
        with (
            tc.tile_pool(name="io", bufs=2) as iop,
            tc.tile_pool(name="idx", bufs=2) as idxp,
            tc.tile_pool(name="canvas", bufs=1) as canp,
            tc.tile_pool(name="ot", bufs=4) as otp,
            tc.tile_pool(name="const", bufs=1) as constp,
            tc.tile_pool(name="psum", bufs=2, space="PSUM") as psump,
        ):
            ident = constp.tile([P, P], mybir.dt.float32)
            make_identity(nc, ident[:])
            zsrc = constp.tile([P, 2976], mybir.dt.float32)
            nc.vector.memset(zsrc[:], 0.0)

            # one extra dump group (g=93) receives the zero-valued padding
            # tokens: a padding RMW racing a real token's add on the same
            # cell can lose the real update, so pads must alias nothing real
            canvases = []
            for bu in range(2):
                Ab = canp.tile([P, (JBLK + 1) * C], mybir.dt.float32, tag=f"A{bu}")
                nc.vector.memset(Ab[:], 0.0)
                canvases.append(Ab)

            for ci in range(NCHUNK):
                A = canvases[ci % 2]
                # ft/it ride the software Pool queue: the scatter that reads
                # them is on the same FIFO queue, so the RAW dependency can
                # never be unblocked early by another queue's completions
                # (the 8 DMAHW/DMASW sem lanes are shared count-based sems).
                ft = iop.tile([P, jr, C], mybir.dt.float32, tag="ft")
                nc.gpsimd.dma_start(ft[:], feats[ci * nslot:(ci + 1) * nslot, :].rearrange("(p j) c -> p j c", p=P))
                it = idxp.tile([P, nslot // 16], mybir.dt.int16, tag="it")
                nc.gpsimd.dma_start(it[:], sidx[:, ci * (nslot // 16):(ci + 1) * (nslot // 16)])

                nc.gpsimd.dma_scatter_add(
                    out_ap=A[:], in_ap=ft[:], idxs_ap=it[:],
                    num_idxs=nslot, num_idxs_reg=nslot, elem_size=C,
                    single_packet=False, sbuf_tokens_per_rank=P,
                    parity_reg=0, out_ap_other=A[:],
                )

                # Canvas block order is host-permuted: canvas block 2t = plane
                # block t (first half-chunk), canvas block 2t+1 = plane block
                # 47+t (second half).  Pair-transposes then put the first
                # half-chunk's channels in PSUM rows 0:64 and the second
                # half's in rows 64:128, so the PSUM->SBUF copies are a
                # single full-width [128, *] copy per PSUM tile, and the two
                # output DMAs read fully-contiguous [64, *] lines.
                T = otp.tile([P, 47 * P], mybir.dt.float32, tag="ot")
                for t in range(3):
                    p0 = t * 16                       # first pair of this tile
                    npair = min(16, 46 - p0)          # pairs in tile (16/16/14)
                    pt = psump.tile([P, 2048], mybir.dt.float32, tag="pt")
                    for m in range(npair):
                        g0 = 2 * (p0 + m)
                        nc.tensor.transpose(pt[:, m * P:(m + 1) * P], A[:, g0 * C:(g0 + 2) * C], ident[:])
                    if t == 2:
                        # canvas block 92 (plane block 46, first half) alone
                        nc.tensor.transpose(pt[0:C, npair * P:(npair + 1) * P], A[:, 92 * C:93 * C], ident[:])
                    if t == 0:
                        nc.vector.tensor_copy(T[:, p0 * P:(p0 + npair) * P], pt[:, 0:npair * P])
                    else:
                        nc.scalar.copy(T[:, p0 * P:(p0 + npair) * P], pt[:, 0:npair * P])
                    if t == 2:
                        nc.vector.tensor_copy(T[0:C, 46 * P:47 * P], pt[0:C, npair * P:(npair + 1) * P])

                # out: first half-chunk (47 blocks) from rows 0:64 on the SP
                # queue, second half (46 blocks) from rows 64:128 on ACT
                nc.sync.dma_start(out_flat[:, ci * MC: ci * MC + 47 * P], T[0:C, :])
                nc.scalar.dma_start(out_flat[:, ci * MC + 47 * P:(ci + 1) * MC], T[C:P, 0:46 * P])

                # re-zero the canvas for chunk ci+2 (split across engines;
                # the dump group only ever accumulates zeros, skip it)
                nc.vector.memset(A[:, 0:2976], 0.0)
                nc.scalar.copy(A[:, 2976:JBLK * C], zsrc[:])

    nc.finalize()
    return nc


def _prep_in_maps(feats_full, batch_indices, sample_indices):
    x = batch_indices[:, 2].astype(np.int64)
    y = batch_indices[:, 1].astype(np.int64)
    sm = sample_indices.astype(np.int64)
    xo = (NX - 1) - x
    h = xo // XH
    xl = xo % XH
    pos = xl * NY + y
    core = sm * 2 + h

    ci = pos // MC                  # chunk
    local = pos % MC
    jpl = local // P                # plane block within chunk (0..92)
    g = np.where(jpl < 47, 2 * jpl, 2 * (jpl - 47) + 1)   # canvas block
    idx16 = g * 256 + (local % P)

    grp = core * NCHUNK + ci
    counts = np.bincount(grp, minlength=NCORES * NCHUNK)
    maxn = int(counts.max())
    jr = -(-maxn // P)
    nslot = P * jr

    order = np.argsort(grp, kind="stable")
    in_maps = []
    off = 0
    for k in range(NCORES):
        feats_arr = np.zeros((NCHUNK * nslot, C), np.float32)
        idx_arr = np.zeros((P, NCHUNK * nslot // 16), np.int16)
        for g in range(NCHUNK):
            n = counts[k * NCHUNK + g]
            rows = order[off:off + n]
            off += n
            # Issue tokens rotating across partitions in STRIDE4_SWIZZLE
            # order: consecutive in-flight scatter packets hit all 4 SBUF
            # write ports AND never target the same partition back-to-back
            # (concurrent CCE adds on one partition can collide).
            pp = local[rows] % P
            cls_order = np.argsort(pp, kind="stable")
            rank = np.empty(n, np.int64)
            pcounts = np.bincount(pp, minlength=P)
            start = 0
            for q in range(P):
                rank[cls_order[start:start + pcounts[q]]] = np.arange(pcounts[q])
                start += pcounts[q]
            rows = rows[np.argsort(rank * P + _SWZ_POS[pp], kind="stable")]
            slots = np.arange(n)
            allslots = np.arange(nslot)
            vals = (JBLK * 256 + allslots % P).astype(np.int16)   # pads -> dump group
            vals[:n] = idx16[rows].astype(np.int16)
            d = (slots % P) * jr + slots // P
            feats_arr[g * nslot + d] = feats_full[rows]
            idx_arr[:16, g * (nslot // 16):(g + 1) * (nslot // 16)] = vals.reshape(nslot // 16, 16).T
        idx_arr[16:] = np.tile(idx_arr[:16], (7, 1))
        in_maps.append({"feats": feats_arr, "sidx": idx_arr})
    return in_maps, jr


def kernel(batch_pillar_features, batch_indices, sample_indices, batch_size):
    global LAST_RESULTS
    feats_full = np.asarray(batch_pillar_features, np.float32)
    batch_indices = np.asarray(batch_indices)
    sample_indices = np.asarray(sample_indices)
    bs = int(batch_size)
    assert bs == B and feats_full.shape[1] == C

    in_maps, jr = _prep_in_maps(feats_full, batch_indices, sample_indices)
    if _CACHE.get("jr") != jr:
        _CACHE["nc"] = _build_program(jr)
        _CACHE["jr"] = jr
    nc = _CACHE["nc"]

    res = run_bass_kernel_spmd(nc, in_maps, core_ids=list(range(NCORES)))
    LAST_RESULTS = res

    full = np.empty((B, C, NX, NY), np.float32)
    for k in range(NCORES):
        b, hh = k // 2, k % 2
        full[b, :, hh * XH:(hh + 1) * XH, :] = res.results[k]["out"]
    return full
